# revision 31
# baseline (speedup 1.0000x reference)
"""LM-Infinite sparse attention kernel for Trainium2 (8 NeuronCores), v3.

Reference semantics: causal attention with additive bias min(j-i, 2048) on
logits, masked to keys j in [0, n_global) U [i-2047, i].  Because the bias
decays as e^(j-i), in f32 the output equals sliding-window attention with a
~91-key window; in our fp16 pipeline the host-precomputed bias e^(j-i)
underflows at distance >= 18, so the previous-block contribution only
matters for the first 32 queries of each 128-query tile.

Per 128-query tile t: keys from the diagonal block plus the first 32
queries' view of the previous block.  Everything is computed transposed
(ST[j,q]) so P^T feeds the PV matmul directly and V needs no transpose.
Softmax runs without row-max (logits are small); the kernel returns raw
numerators + denominators per tile and the host divides.

Trace-driven structure (vs the 28.4us v2 baseline; ~24.4us measured):
 - ST blocks are 160 query-cols (diag 128 + prev 32); the prev sections in
   pt are zero-padded to 128 so every PV close matmul uses a full 128-wide
   stationary (mixed PE tile configs broke back-to-back matmul pipelining).
 - quad-granular pipeline: 4 key blocks share one [128,1024] PSUM pair, one
   strided [128,4,160] exp and one bias-mul (bias broadcast stride-0, so
   the bias input is a single [128,160] tile).
 - PE pre-warm matmuls bridge the DMA latency so the Tensor engine's DVFS
   ramp (0.65/1.2 -> 2.4GHz after ~3us continuous busy) is done early and
   idle gaps (which reset the ramp) are minimized.
 - coarse 2-chunk loads (split at the quad-2 boundary): per-queue DMA
   bandwidth scales with descriptor size (per-partition row bytes) -- 1KB
   rows gave ~130GB/s/queue, 3.3KB rows ~270GB/s/queue, ~345GB/s aggregate
   across the three queues.  Q rides SP, K rides the ACT HWDGE queue,
   bias+V ride SWDGE; evacs split DVE/ACT; the final store chunk is small.
 - the TileContext end-block drops its RANGE_CLEAR + second barrier: the
   NEFF-level epilogue re-zeroes every semaphore (S[2..255], one insn per
   sem -- a fixed ~7us cost on every kernel) right after it anyway.

Sharding: core = b*4 + cc handles batch b, queries [cc*2048, (cc+1)*2048).
K/V carry a 128-key halo; chunk-0 cores get an all-zero halo V block
(including its ones-column) so the halo contributes nothing.
"""

import math
import types
import numpy as np

import concourse.bass as bass
import concourse.mybir as mybir
import concourse.tile as tile
from concourse import bacc
from concourse.bass_utils import run_bass_kernel_spmd
from concourse.vector_clock import ScopedClock

B, S, D = 2, 8192, 128
NCORES = 8
CHUNK = S // 4          # 2048 queries per core
NQT = CHUNK // 128      # 16 query tiles per core
NKB = NQT + 1           # 17 key blocks incl. halo
NPAIR = 8               # 8 pairs of key blocks (blocks 1..16)
PRE = 32                # prev-section query cols (bias==0 beyond dist 17)
DW = 128 + PRE          # 160: per-block section width (diag 128 | prev 32)
SEC = 256               # PSUM section stride (f32 bank-aligned)
VW = 129                # V block width incl. ones-column
VNW = NKB * VW          # 2193
OBW = NQT * VW          # 2064 output cols (128 num + 1 den per tile)
F16 = mybir.dt.float16
F32 = mybir.dt.float32
SCALE = 1.0 / math.sqrt(D)
NWARM = 16              # PE pre-warm matmuls
LEAN_END = True

_CACHE = {}


def _lean_drain_and_barrier(self, tick_clock, wait_clock):
    # Keep the store-completion waits; rendezvous ONLY the engines whose
    # NEFF-epilogue sweep ranges contain live semaphores (SP waits the
    # stores; Pool's range holds the barrier/tile sems; DVE's range holds
    # the DMA sems).  PE (S[2..53]) and ACT (S[54..104]) sweep only
    # walrus-internal sems that are quiescent once they finish computing,
    # so releasing them early overlaps their ~5-6us sweep chains with the
    # store drain.  (Skip the RANGE_CLEAR + second barrier entirely: the
    # epilogue re-zeroes every semaphore anyway.)
    drain_inst = self.nc.sync.drain()
    wait_clock.add_sem_waits(
        drain_inst.ins, ScopedClock({None: tick_clock.global_clock})
    )
    self.nc.multi_engine_barrier(
        [mybir.EngineType.SP, mybir.EngineType.Pool, mybir.EngineType.DVE]
    )
    popped = self.nc._tile_sem_poison_stack.pop()
    assert popped is self._sem_poison


def _build_bass():
    nc = bacc.Bacc("TRN2", target_bir_lowering=False, debug=False)
    qt_d = nc.dram_tensor("qt", [128, CHUNK], F16, kind="ExternalInput").ap()
    kt_d = nc.dram_tensor("kt", [128, NKB * 128], F16,
                          kind="ExternalInput").ap()
    vn_d = nc.dram_tensor("vn", [128, VNW], F16, kind="ExternalInput").ap()
    bias_d = nc.dram_tensor("bias", [128, DW], F16, kind="ExternalInput").ap()
    out = nc.dram_tensor("out", [128, OBW], F16, kind="ExternalOutput").ap()

    with tile.TileContext(nc) as tc:
        if LEAN_END:
            tc._drain_and_barrier = types.MethodType(_lean_drain_and_barrier,
                                                     tc)
        with (
            tc.tile_pool(name="big", bufs=1) as big,
            tc.tile_pool(name="ptp", bufs=4) as ptp,
            tc.tile_pool(name="ppp", bufs=3) as ppp,
            tc.tile_pool(name="stq", bufs=2, space="PSUM") as stq,
            tc.tile_pool(name="otp", bufs=4, space="PSUM") as otp,
        ):
            QT = big.tile([128, CHUNK], F16)
            KT = big.tile([128, NKB * 128], F16)
            VN = big.tile([128, VNW], F16)
            BT = big.tile([128, DW], F16)
            OB = big.tile([128, OBW], F16)
            WT = big.tile([128, 256], F16)

            # --- loads: fine-grained need order, K/Q alternating across
            # the two HWDGE queues, V+bias on SWDGE ----------------------
            nc.gpsimd.memset(WT[:], 0)
            nc.sync.dma_start(QT[:, 0:1568], qt_d[:, 0:1568])
            nc.scalar.dma_start(KT[:, 0:1664], kt_d[:, 0:1664])
            nc.gpsimd.dma_start(BT[:], bias_d[:])
            nc.gpsimd.dma_start(VN[:, 0:1677], vn_d[:, 0:1677])
            nc.sync.dma_start(QT[:, 1568:2048], qt_d[:, 1568:2048])
            nc.scalar.dma_start(KT[:, 1664:2176], kt_d[:, 1664:2176])
            nc.gpsimd.dma_start(VN[:, 1677:VNW], vn_d[:, 1677:VNW])

            # --- PE pre-warm ------------------------------------------------
            warm = otp.tile([128, 256], F32, tag="ot", name="warm")
            for _ in range(NWARM):
                nc.tensor.matmul(warm[:], WT[:, 0:128], WT[:, 0:256],
                                 start=True, stop=True)

            # --- halo block 0: prev-only for tile 0 -------------------------
            sth = otp.tile([128, PRE], F32, tag="ot", name="sth")
            nc.tensor.matmul(sth[:], KT[:, 0:128], QT[:, 0:PRE],
                             start=True, stop=True)
            pph = ppp.tile([128, PRE], F16, tag="pp", name="pph")
            nc.scalar.activation(pph[:], sth[:],
                                 mybir.ActivationFunctionType.Exp, scale=SCALE)
            pth = big.tile([128, 128], F16)
            nc.gpsimd.memset(pth[:, PRE:128], 0)
            nc.gpsimd.tensor_mul(pth[:, 0:PRE], pph[:], BT[:, 128:DW])

            pts = {-1: pth}   # pair index -> pt tile (halo at -1)

            def emit_st(q):
                """ST matmuls for quad q (blocks 4q+1 .. 4q+4) into one
                [128,1024] PSUM pair, then exp + bias-mul into pt."""
                st = stq.tile([128, 1024], F32, tag="st", name=f"st{q}")
                nfull = 3 if q == 3 else 4
                for i in range(4):
                    k = 4 * q + 1 + i
                    w = DW if i < nfull else 128
                    nc.tensor.matmul(st[:, SEC * i:SEC * i + w],
                                     KT[:, k * 128:(k + 1) * 128],
                                     QT[:, (k - 1) * 128:(k - 1) * 128 + w],
                                     start=True, stop=True)
                pp = ppp.tile([128, 4 * DW], F16, tag="pp", name=f"pp{q}")
                pt = ptp.tile([128, 1024], F16, tag="pt", name=f"pt{q}")
                st3 = st[:, :].rearrange("p (b c) -> p b c", c=SEC)
                pp3 = pp[:, :].rearrange("p (b c) -> p b c", c=DW)
                pt3 = pt[:, :].rearrange("p (b c) -> p b c", c=SEC)
                bt3 = BT[:, :].unsqueeze(1)
                # zero the prev-section padding so close matmuls can use
                # full 128-wide stationaries (uniform PE tile config)
                nc.gpsimd.memset(pt3[:, :, DW:SEC], 0)
                if nfull == 4:
                    nc.scalar.activation(pp3, st3[:, :, 0:DW],
                                         mybir.ActivationFunctionType.Exp,
                                         scale=SCALE)
                    nc.vector.tensor_mul(pt3[:, :, 0:DW], pp3,
                                         bt3.broadcast_to((128, 4, DW)))
                else:
                    nc.scalar.activation(pp3[:, 0:3], st3[:, 0:3, 0:DW],
                                         mybir.ActivationFunctionType.Exp,
                                         scale=SCALE)
                    nc.scalar.activation(pp[:, 3 * DW:3 * DW + 128],
                                         st[:, 3 * SEC:3 * SEC + 128],
                                         mybir.ActivationFunctionType.Exp,
                                         scale=SCALE)
                    nc.vector.tensor_mul(pt3[:, 0:3, 0:DW], pp3[:, 0:3],
                                         bt3.broadcast_to((128, 3, DW)))
                    nc.vector.tensor_mul(pt[:, 3 * SEC:3 * SEC + 128],
                                         pp[:, 3 * DW:3 * DW + 128],
                                         BT[:, 0:128])
                pts[q] = pt

            def emit_pv(q):
                """PV matmuls + evacs for tiles 4q..4q+3."""
                pt = pts[q]
                for half in range(2):
                    ot = otp.tile([128, 2 * VW], F32, tag="ot",
                                  name=f"ot{q}_{half}")
                    for sub in range(2):
                        t = 4 * q + 2 * half + sub
                        i = t % 4
                        dst = ot[:, sub * VW:(sub + 1) * VW]
                        # diag: block t+1 = section i of quad q
                        nc.tensor.matmul(
                            dst, pt[:, SEC * i:SEC * i + 128],
                            VN[:, (t + 1) * VW:(t + 2) * VW],
                            start=True, stop=False, skip_group_check=True)
                        # prev: block t = section i-1 of quad q (or the
                        # previous quad's last section / the halo)
                        if i == 0:
                            ppt = pts[q - 1]
                            lhs = (ppt[:] if q == 0 else
                                   ppt[:, SEC * 3 + 128:SEC * 4])
                        else:
                            lhs = pt[:, SEC * (i - 1) + 128:SEC * i]
                        nc.tensor.matmul(
                            dst, lhs, VN[:, t * VW:(t + 1) * VW],
                            start=False, stop=True, skip_group_check=True)
                    c0 = 516 * q + 258 * half
                    if half == 0:
                        nc.vector.tensor_copy(OB[:, c0:c0 + 258], ot[:])
                    else:
                        nc.scalar.copy(OB[:, c0:c0 + 258], ot[:])

            # software pipeline: STs run one quad ahead of PVs
            emit_st(0)
            emit_st(1)
            emit_pv(0)
            emit_st(2)
            emit_pv(1)
            nc.sync.dma_start(out[:, 0:1032], OB[:, 0:1032])
            emit_st(3)
            emit_pv(2)
            nc.sync.dma_start(out[:, 1032:1548], OB[:, 1032:1548])
            emit_pv(3)
            nc.sync.dma_start(out[:, 1548:2064], OB[:, 1548:2064])

    nc.compile()
    return nc


def _bias_tile() -> np.ndarray:
    jj = np.arange(128, dtype=np.float64)[:, None]
    uu = np.arange(128, dtype=np.float64)[None, :]
    diag = np.where(jj <= uu, np.exp(jj - uu), 0.0)
    prev = np.exp(jj - 128 - uu[:, :PRE])
    return np.concatenate([diag, prev], axis=1).astype(np.float16)  # [128,160]


def kernel(q: np.ndarray, k: np.ndarray, v: np.ndarray) -> np.ndarray:
    return _run(q, k, v)[0]


def _run(q, k, v, trace=False, tmpdir=None):
    if "nc" not in _CACHE:
        _CACHE["nc"] = _build_bass()
        _CACHE["bias"] = _bias_tile()
    nc = _CACHE["nc"]

    in_maps = []
    for core in range(NCORES):
        b, cc = divmod(core, 4)
        lo, hi = cc * CHUNK, (cc + 1) * CHUNK
        if cc == 0:
            pad = np.zeros((128, D), dtype=np.float32)
            ks = np.concatenate([pad, np.asarray(k[b, lo:hi])], axis=0)
            vs = np.concatenate([pad, np.asarray(v[b, lo:hi])], axis=0)
        else:
            ks = np.asarray(k[b, lo - 128:hi])
            vs = np.asarray(v[b, lo - 128:hi])
        vn = np.zeros((128, VNW), dtype=np.float16)
        vn3 = vn.reshape(128, NKB, VW)
        vn3[:, :, 0:128] = vs.reshape(NKB, 128, D).transpose(1, 0, 2)
        vn3[:, :, 128] = 1.0
        if cc == 0:
            # Neutralize the (nonexistent) halo block: zero its ones-column
            # so it contributes nothing to numerator or denominator.
            vn3[:, 0, 128] = 0.0
        in_maps.append({
            "qt": np.ascontiguousarray(np.asarray(q[b, lo:hi]).T
                                       ).astype(np.float16),
            "kt": np.ascontiguousarray(ks.T).astype(np.float16),
            "vn": vn,
            "bias": _CACHE["bias"],
        })

    res = run_bass_kernel_spmd(nc, in_maps, list(range(NCORES)),
                               trace=trace, tmpdir=tmpdir)
    out = np.empty((B, S, D), dtype=np.float32)
    for core in range(NCORES):
        b, cc = divmod(core, 4)
        ob = res.results[core]["out"].astype(np.float32)  # [128, 2064]
        for t in range(NQT):
            num = ob[:, t * VW:t * VW + 128]
            den = ob[:, t * VW + 128:t * VW + 129]
            out[b, cc * CHUNK + t * 128:cc * CHUNK + (t + 1) * 128] = num / den
    return out, res


# revision 32
# speedup vs baseline: 1.1551x; 1.1551x over previous
"""LM-Infinite sparse attention kernel for Trainium2 (8 NeuronCores), v3.

Reference semantics: causal attention with additive bias min(j-i, 2048) on
logits, masked to keys j in [0, n_global) U [i-2047, i].  Because the bias
decays as e^(j-i), in f32 the output equals sliding-window attention with a
~91-key window; in our fp16 pipeline the host-precomputed bias e^(j-i)
underflows at distance >= 18, so the previous-block contribution only
matters for the first 32 queries of each 128-query tile.

Per 128-query tile t: keys from the diagonal block plus the first 32
queries' view of the previous block.  Everything is computed transposed
(ST[j,q]) so P^T feeds the PV matmul directly and V needs no transpose.
Softmax runs without row-max (logits are small); the kernel returns raw
numerators + denominators per tile and the host divides.

Trace-driven structure (vs the 28.4us v2 baseline; ~24.4us measured):
 - ST blocks are 160 query-cols (diag 128 + prev 32); the prev sections in
   pt are zero-padded to 128 so every PV close matmul uses a full 128-wide
   stationary (mixed PE tile configs broke back-to-back matmul pipelining).
 - quad-granular pipeline: 4 key blocks share one [128,1024] PSUM pair, one
   strided [128,4,160] exp and one bias-mul (bias broadcast stride-0, so
   the bias input is a single [128,160] tile).
 - PE pre-warm matmuls bridge the DMA latency so the Tensor engine's DVFS
   ramp (0.65/1.2 -> 2.4GHz after ~3us continuous busy) is done early and
   idle gaps (which reset the ramp) are minimized.
 - coarse 2-chunk loads (split at the quad-2 boundary): per-queue DMA
   bandwidth scales with descriptor size (per-partition row bytes) -- 1KB
   rows gave ~130GB/s/queue, 3.3KB rows ~270GB/s/queue, ~345GB/s aggregate
   across the three queues.  Q rides SP, K rides the ACT HWDGE queue,
   bias+V ride SWDGE; evacs split DVE/ACT; the final store chunk is small.
 - the TileContext end-block drops its RANGE_CLEAR + second barrier: the
   NEFF-level epilogue re-zeroes every semaphore (S[2..255], one insn per
   sem -- a fixed ~7us cost on every kernel) right after it anyway.

Sharding: core = b*4 + cc handles batch b, queries [cc*2048, (cc+1)*2048).
K/V carry a 128-key halo; chunk-0 cores get an all-zero halo V block
(including its ones-column) so the halo contributes nothing.
"""

import math
import types
import numpy as np

import concourse.bass as bass
import concourse.mybir as mybir
import concourse.tile as tile
from concourse import bacc
from concourse.bass_utils import run_bass_kernel_spmd
from concourse.vector_clock import ScopedClock

B, S, D = 2, 8192, 128
NCORES = 8
CHUNK = S // 4          # 2048 queries per core
NQT = CHUNK // 128      # 16 query tiles per core
NKB = NQT + 1           # 17 key blocks incl. halo
NPAIR = 8               # 8 pairs of key blocks (blocks 1..16)
PRE = 32                # prev-section query cols (bias==0 beyond dist 17)
DW = 128 + PRE          # 160: per-block section width (diag 128 | prev 32)
SEC = 256               # PSUM section stride (f32 bank-aligned)
VW = 129                # V block width incl. ones-column
VNW = NKB * VW          # 2193
OBW = NQT * VW          # 2064 output cols (128 num + 1 den per tile)
F16 = mybir.dt.float16
F32 = mybir.dt.float32
SCALE = 1.0 / math.sqrt(D)
NWARM = 16              # PE pre-warm matmuls
LEAN_END = True

_CACHE = {}


def _lean_drain_and_barrier(self, tick_clock, wait_clock):
    # Keep the store-completion waits; rendezvous ONLY the engines whose
    # NEFF-epilogue sweep ranges contain live semaphores (SP waits the
    # stores; Pool's range holds the barrier/tile sems; DVE's range holds
    # the DMA sems).  PE (S[2..53]) and ACT (S[54..104]) sweep only
    # walrus-internal sems that are quiescent once they finish computing,
    # so releasing them early overlaps their ~5-6us sweep chains with the
    # store drain.  (Skip the RANGE_CLEAR + second barrier entirely: the
    # epilogue re-zeroes every semaphore anyway.)
    # No barrier at all: the NEFF epilogue's own S[2] rendezvous (every
    # engine waits $S[2]==8 before its sweep -- trace-verified) is the
    # global sync; SP's drain carries all store-completion waits, so no
    # engine can zero a live semaphore early.
    drain_inst = self.nc.sync.drain()
    wait_clock.add_sem_waits(
        drain_inst.ins, ScopedClock({None: tick_clock.global_clock})
    )
    popped = self.nc._tile_sem_poison_stack.pop()
    assert popped is self._sem_poison


def _build_bass():
    nc = bacc.Bacc("TRN2", target_bir_lowering=False, debug=False)
    qt_d = nc.dram_tensor("qt", [128, CHUNK], F16, kind="ExternalInput").ap()
    kt_d = nc.dram_tensor("kt", [128, NKB * 128], F16,
                          kind="ExternalInput").ap()
    vn_d = nc.dram_tensor("vn", [128, VNW], F16, kind="ExternalInput").ap()
    bias_d = nc.dram_tensor("bias", [128, DW], F16, kind="ExternalInput").ap()
    out = nc.dram_tensor("out", [128, OBW], F16, kind="ExternalOutput").ap()

    with tile.TileContext(nc) as tc:
        if LEAN_END:
            tc._drain_and_barrier = types.MethodType(_lean_drain_and_barrier,
                                                     tc)
        with (
            tc.tile_pool(name="big", bufs=1) as big,
            tc.tile_pool(name="ptp", bufs=4) as ptp,
            tc.tile_pool(name="ppp", bufs=3) as ppp,
            tc.tile_pool(name="stq", bufs=2, space="PSUM") as stq,
            tc.tile_pool(name="otp", bufs=4, space="PSUM") as otp,
        ):
            QT = big.tile([128, CHUNK], F16)
            KT = big.tile([128, NKB * 128], F16)
            VN = big.tile([128, VNW], F16)
            BT = big.tile([128, DW], F16)
            OB = big.tile([128, OBW], F16)
            WT = big.tile([128, 256], F16)

            # --- loads: fine-grained need order, K/Q alternating across
            # the two HWDGE queues, V+bias on SWDGE ----------------------
            nc.gpsimd.memset(WT[:], 0)
            nc.sync.dma_start(QT[:, 0:1568], qt_d[:, 0:1568])
            nc.scalar.dma_start(KT[:, 0:1664], kt_d[:, 0:1664])
            nc.gpsimd.dma_start(BT[:], bias_d[:])
            nc.gpsimd.dma_start(VN[:, 0:1677], vn_d[:, 0:1677])
            nc.sync.dma_start(QT[:, 1568:2048], qt_d[:, 1568:2048])
            nc.scalar.dma_start(KT[:, 1664:2176], kt_d[:, 1664:2176])
            nc.gpsimd.dma_start(VN[:, 1677:VNW], vn_d[:, 1677:VNW])

            # --- PE pre-warm ------------------------------------------------
            warm = otp.tile([128, 256], F32, tag="ot", name="warm")
            for _ in range(NWARM):
                nc.tensor.matmul(warm[:], WT[:, 0:128], WT[:, 0:256],
                                 start=True, stop=True)

            # --- halo block 0: prev-only for tile 0 -------------------------
            sth = otp.tile([128, PRE], F32, tag="ot", name="sth")
            nc.tensor.matmul(sth[:], KT[:, 0:128], QT[:, 0:PRE],
                             start=True, stop=True)
            pph = ppp.tile([128, PRE], F16, tag="pp", name="pph")
            nc.scalar.activation(pph[:], sth[:],
                                 mybir.ActivationFunctionType.Exp, scale=SCALE)
            pth = big.tile([128, 128], F16)
            nc.gpsimd.memset(pth[:, PRE:128], 0)
            nc.gpsimd.tensor_mul(pth[:, 0:PRE], pph[:], BT[:, 128:DW])

            pts = {-1: pth}   # pair index -> pt tile (halo at -1)

            def emit_st(q):
                """ST matmuls for quad q (blocks 4q+1 .. 4q+4) into one
                [128,1024] PSUM pair, then exp + bias-mul into pt."""
                st = stq.tile([128, 1024], F32, tag="st", name=f"st{q}")
                nfull = 3 if q == 3 else 4
                for i in range(4):
                    k = 4 * q + 1 + i
                    w = DW if i < nfull else 128
                    nc.tensor.matmul(st[:, SEC * i:SEC * i + w],
                                     KT[:, k * 128:(k + 1) * 128],
                                     QT[:, (k - 1) * 128:(k - 1) * 128 + w],
                                     start=True, stop=True)
                pp = ppp.tile([128, 4 * DW], F16, tag="pp", name=f"pp{q}")
                pt = ptp.tile([128, 1024], F16, tag="pt", name=f"pt{q}")
                st3 = st[:, :].rearrange("p (b c) -> p b c", c=SEC)
                pp3 = pp[:, :].rearrange("p (b c) -> p b c", c=DW)
                pt3 = pt[:, :].rearrange("p (b c) -> p b c", c=SEC)
                bt3 = BT[:, :].unsqueeze(1)
                # zero the prev-section padding so close matmuls can use
                # full 128-wide stationaries (uniform PE tile config)
                nc.gpsimd.memset(pt3[:, :, DW:SEC], 0)
                if nfull == 4:
                    nc.scalar.activation(pp3, st3[:, :, 0:DW],
                                         mybir.ActivationFunctionType.Exp,
                                         scale=SCALE)
                    nc.vector.tensor_mul(pt3[:, :, 0:DW], pp3,
                                         bt3.broadcast_to((128, 4, DW)))
                else:
                    nc.scalar.activation(pp3[:, 0:3], st3[:, 0:3, 0:DW],
                                         mybir.ActivationFunctionType.Exp,
                                         scale=SCALE)
                    nc.scalar.activation(pp[:, 3 * DW:3 * DW + 128],
                                         st[:, 3 * SEC:3 * SEC + 128],
                                         mybir.ActivationFunctionType.Exp,
                                         scale=SCALE)
                    nc.vector.tensor_mul(pt3[:, 0:3, 0:DW], pp3[:, 0:3],
                                         bt3.broadcast_to((128, 3, DW)))
                    nc.vector.tensor_mul(pt[:, 3 * SEC:3 * SEC + 128],
                                         pp[:, 3 * DW:3 * DW + 128],
                                         BT[:, 0:128])
                pts[q] = pt

            def emit_pv(q):
                """PV matmuls + evacs for tiles 4q..4q+3."""
                pt = pts[q]
                for half in range(2):
                    ot = otp.tile([128, 2 * VW], F32, tag="ot",
                                  name=f"ot{q}_{half}")
                    for sub in range(2):
                        t = 4 * q + 2 * half + sub
                        i = t % 4
                        dst = ot[:, sub * VW:(sub + 1) * VW]
                        # diag: block t+1 = section i of quad q
                        nc.tensor.matmul(
                            dst, pt[:, SEC * i:SEC * i + 128],
                            VN[:, (t + 1) * VW:(t + 2) * VW],
                            start=True, stop=False, skip_group_check=True)
                        # prev: block t = section i-1 of quad q (or the
                        # previous quad's last section / the halo)
                        if i == 0:
                            ppt = pts[q - 1]
                            lhs = (ppt[:] if q == 0 else
                                   ppt[:, SEC * 3 + 128:SEC * 4])
                        else:
                            lhs = pt[:, SEC * (i - 1) + 128:SEC * i]
                        nc.tensor.matmul(
                            dst, lhs, VN[:, t * VW:(t + 1) * VW],
                            start=False, stop=True, skip_group_check=True)
                    c0 = 516 * q + 258 * half
                    if half == 0:
                        nc.vector.tensor_copy(OB[:, c0:c0 + 258], ot[:])
                    else:
                        nc.scalar.copy(OB[:, c0:c0 + 258], ot[:])

            # software pipeline: STs run one quad ahead of PVs
            emit_st(0)
            emit_st(1)
            emit_pv(0)
            emit_st(2)
            emit_pv(1)
            nc.sync.dma_start(out[:, 0:1032], OB[:, 0:1032])
            emit_st(3)
            emit_pv(2)
            nc.sync.dma_start(out[:, 1032:1548], OB[:, 1032:1548])
            emit_pv(3)
            nc.sync.dma_start(out[:, 1548:2064], OB[:, 1548:2064])

    nc.compile()
    return nc


def _bias_tile() -> np.ndarray:
    jj = np.arange(128, dtype=np.float64)[:, None]
    uu = np.arange(128, dtype=np.float64)[None, :]
    diag = np.where(jj <= uu, np.exp(jj - uu), 0.0)
    prev = np.exp(jj - 128 - uu[:, :PRE])
    return np.concatenate([diag, prev], axis=1).astype(np.float16)  # [128,160]


def kernel(q: np.ndarray, k: np.ndarray, v: np.ndarray) -> np.ndarray:
    return _run(q, k, v)[0]


def _run(q, k, v, trace=False, tmpdir=None):
    if "nc" not in _CACHE:
        _CACHE["nc"] = _build_bass()
        _CACHE["bias"] = _bias_tile()
    nc = _CACHE["nc"]

    in_maps = []
    for core in range(NCORES):
        b, cc = divmod(core, 4)
        lo, hi = cc * CHUNK, (cc + 1) * CHUNK
        if cc == 0:
            pad = np.zeros((128, D), dtype=np.float32)
            ks = np.concatenate([pad, np.asarray(k[b, lo:hi])], axis=0)
            vs = np.concatenate([pad, np.asarray(v[b, lo:hi])], axis=0)
        else:
            ks = np.asarray(k[b, lo - 128:hi])
            vs = np.asarray(v[b, lo - 128:hi])
        vn = np.zeros((128, VNW), dtype=np.float16)
        vn3 = vn.reshape(128, NKB, VW)
        vn3[:, :, 0:128] = vs.reshape(NKB, 128, D).transpose(1, 0, 2)
        vn3[:, :, 128] = 1.0
        if cc == 0:
            # Neutralize the (nonexistent) halo block: zero its ones-column
            # so it contributes nothing to numerator or denominator.
            vn3[:, 0, 128] = 0.0
        in_maps.append({
            "qt": np.ascontiguousarray(np.asarray(q[b, lo:hi]).T
                                       ).astype(np.float16),
            "kt": np.ascontiguousarray(ks.T).astype(np.float16),
            "vn": vn,
            "bias": _CACHE["bias"],
        })

    res = run_bass_kernel_spmd(nc, in_maps, list(range(NCORES)),
                               trace=trace, tmpdir=tmpdir)
    out = np.empty((B, S, D), dtype=np.float32)
    for core in range(NCORES):
        b, cc = divmod(core, 4)
        ob = res.results[core]["out"].astype(np.float32)  # [128, 2064]
        for t in range(NQT):
            num = ob[:, t * VW:t * VW + 128]
            den = ob[:, t * VW + 128:t * VW + 129]
            out[b, cc * CHUNK + t * 128:cc * CHUNK + (t + 1) * 128] = num / den
    return out, res
